# revision 14
# baseline (speedup 1.0000x reference)
"""Causal self-attention (B=8, T=1024, C=768, H=8 heads) for 8 TRN2 NeuronCores.

Strategy: pure data parallelism — one batch element per core. Each core runs an
identical Bass/Tile program computing the full attention block for its batch
element; weights are replicated. No collectives.

v3 (from trace analysis of v2, 164.0us):
  - Host-provided constants (identity / triangle mask / ones, and the q,k bias
    columns pre-rearranged): removes the gpsimd setup chain that delayed the
    first tensor op to 7.8us, and the 1536-descriptor bias scatter DMA.
  - PE warm-up matmuls start right after the constant DMA (~0.5us in), so the
    HAM clock gate (K=4/8 -> 1.2 GHz cold) opens before the real work.
  - Per-head software pipeline: the 8 S^T blocks of head h are interleaved
    with the q/k projection matmuls of head h+1, so the serial exp chain on
    the ACT engine (~6us/head) hides under PE work instead of pacing it
    (v2 ran a 12.7us/head cadence for 9.4us of PE work).
  - Softmax tails divide by the denominator on the DVE (AluOpType.divide)
    instead of Ln/Exp reciprocal: removes 32 ACT ops (~24us) from the
    scalar engine, which was co-saturated with exp.
  - Heads 6/7 broadcast the denominator across partitions with a ones-column
    PE matmul instead of the DRAM round trip; head 6's tail gates the
    early output-projection blocks emitted inside head 7's cycle.
  - Output projection split: contraction blocks cb0-4 of token blocks 0-3 run
    inside head 7's cycle (covering its exp chain); the cb5 contribution
    (which needs head 7's result) lands in a second accumulation pass.

Per-core pipeline:
  1. x [T,C] -> x^T [C,T] via PE transposes, bf16.
  2. v = x @ W_v + b_v in [token, feat] layout as v_aug [128, 8*97] with a
     ones column per head (the P@V matmul then also produces the softmax
     denominator).
  3. Per head: q^T, k^T = (x @ W_{q,k} + b)^T [96, 1024]; S^T[tk,q] =
     k^T.T @ q^T per 128-key block over the causally-valid query range;
     P = exp(S*scale) masked by a DVE triangle multiply; y_aug^T = V_aug^T P.
  4. y^T = y_aug^T[0:96] / broadcast(denominator); out = y @ W_proj + b_proj.
"""
import sys

sys.path.insert(0, "/opt/trn_rl_repo")

import numpy as np

T, C, H, D = 1024, 768, 8, 96
C3 = 3 * C
P = 128
NT = T // P   # 8 token blocks
NCB = C // P  # 6 feature blocks
DA = D + 1    # 97: head dim + denominator column

_CACHE = {}


def _build():
    import concourse.bacc as bacc
    import concourse.mybir as mybir
    import concourse.tile as tile

    F32 = mybir.dt.float32
    BF16 = mybir.dt.bfloat16
    Exp = mybir.ActivationFunctionType.Exp
    Ln = mybir.ActivationFunctionType.Ln
    SCALE = 1.0 / float(np.sqrt(D))

    nc = bacc.Bacc("TRN2", target_bir_lowering=False, debug=False, num_devices=8)
    x_d = nc.dram_tensor("x", [T, C], BF16, kind="ExternalInput").ap()
    wa_d = nc.dram_tensor("W_attn", [C, C3], BF16, kind="ExternalInput").ap()
    ba_d = nc.dram_tensor("b_attn", [C3], F32, kind="ExternalInput").ap()
    wp_d = nc.dram_tensor("W_proj", [C, C], BF16, kind="ExternalInput").ap()
    bp_d = nc.dram_tensor("b_proj", [C], F32, kind="ExternalInput").ap()
    # host-prepared constants: [identity | triangle mask | ones]
    auxc_d = nc.dram_tensor("aux_c", [P, 3 * P], BF16, kind="ExternalInput").ap()
    auxb_d = nc.dram_tensor("aux_b", [D, 16], F32, kind="ExternalInput").ap()
    out_d = nc.dram_tensor("out", [T, C], F32, kind="ExternalOutput").ap()

    with tile.TileContext(nc) as tc:
        with tc.tile_pool(name="const", bufs=1) as const_p, \
             tc.tile_pool(name="vp", bufs=1) as v_p, \
             tc.tile_pool(name="qkt", bufs=5) as qk_p, \
             tc.tile_pool(name="yt", bufs=1) as yT_p, \
             tc.tile_pool(name="sm", bufs=2) as sm_p, \
             tc.tile_pool(name="ob", bufs=2) as o_p, \
             tc.tile_pool(name="pp", bufs=8) as p_p, \
             tc.tile_pool(name="ps", bufs=1, space="PSUM") as ps:
            constF = const_p.tile([P, 3 * P], BF16, name="constF")
            ident = constF[:, 0:P]
            tri = constF[:, P:2 * P]
            ones8_f = constF[:, 2 * P:2 * P + H]
            ones97 = constF[0:1, 2 * P + H:2 * P + H + DA]
            b_qk = const_p.tile([D, 16], F32, name="b_qk")
            nc.sync.dma_start(constF[:], auxc_d)
            nc.sync.dma_start(b_qk[:], auxb_d)
            bv_bc = const_p.tile([P, C], F32, name="bv_bc")
            vA = [v_p.tile([P, DA * H], BF16, name=f"vA{t}") for t in range(NT)]
            yTp = [yT_p.tile([P, T], BF16, name=f"yTp{cb}") for cb in range(NCB)]
            # DRAM staging for the denominator rows (SBUF->SBUF DMA cannot do
            # 0-stride broadcast, DRAM->SBUF can); heads 0-5 only
            den_dram = nc.dram_tensor("den_stage", [12, 512], BF16,
                                      kind="Internal").ap()

            with tc.tile_pool(name="xT", bufs=1) as xT_p, \
                 tc.tile_pool(name="wqk", bufs=1) as wqk_p:
                xT = [xT_p.tile([P, T], BF16, name=f"xT{cb}") for cb in range(NCB)]

                # ---- x^T transposes + v projection (scoped W_v / x loads) ----
                wqk = []
                with tc.tile_pool(name="xl", bufs=8) as x_p, \
                     tc.tile_pool(name="wv", bufs=1) as wv_p:
                    # issue every load up front; they stream on parallel DMA
                    # queues while the PE warm-up runs
                    x_ts = []
                    for tb in range(NT):
                        x_t = x_p.tile([P, C], BF16, name="x_t")
                        nc.sync.dma_start(x_t[:], x_d[tb * P:(tb + 1) * P, :])
                        x_ts.append(x_t)
                    wv = []
                    for cb in range(NCB):
                        w = wv_p.tile([P, C], BF16, name=f"wv{cb}")
                        nc.sync.dma_start(w[:], wa_d[cb * P:(cb + 1) * P,
                                                     2 * C:3 * C])
                        wv.append(w)
                    for cb in range(NCB):
                        w = wqk_p.tile([P, 2 * C], BF16, name=f"wqk{cb}")
                        nc.sync.dma_start(w[:], wa_d[cb * P:(cb + 1) * P, 0:2 * C])
                        wqk.append(w)

                    # PE warm-up: the HAM clock gate defaults to K=4/8
                    # (1.2 GHz) and needs ~3.4us of sustained matmul activity
                    # to open. Burn the x-DMA wait on dummy matmuls so the
                    # real transposes start at full clock.
                    warm_ps = ps.tile([P, 2 * P], F32, name="warm_ps",
                                      tag="big", bufs=3)
                    for _ in range(10):
                        nc.tensor.matmul(warm_ps[:], ident, constF[:, 0:2 * P],
                                         start=True, stop=True)

                    for jt in range(2):
                        for cb in range(NCB):
                            tr_ps = ps.tile([P, 512], BF16, name="tr_ps", tag="big", bufs=3)
                            for k in range(4):
                                nc.tensor.transpose(tr_ps[:, k * P:(k + 1) * P],
                                                    x_ts[4 * jt + k][:, cb * P:(cb + 1) * P],
                                                    ident)
                            nc.vector.tensor_copy(xT[cb][:, jt * 512:(jt + 1) * 512],
                                                  tr_ps[:])

                    nc.sync.dma_start(
                        bv_bc[:],
                        ba_d.unsqueeze(0)[:, 2 * C:3 * C].partition_broadcast(P).squeeze(1))
                    for tb in range(NT):
                        v_ps = ps.tile([P, C], F32, name="v_ps", tag="big", bufs=3)
                        for cb in range(NCB):
                            lhsT = xT[cb][:, tb * P:(tb + 1) * P]
                            nc.tensor.matmul(v_ps[:, 0:512], lhsT, wv[cb][:, 0:512],
                                             start=(cb == 0), stop=(cb == NCB - 1))
                            nc.tensor.matmul(v_ps[:, 512:C], lhsT, wv[cb][:, 512:C],
                                             start=(cb == 0), stop=(cb == NCB - 1))
                        for h in range(H):
                            nc.vector.tensor_add(vA[tb][:, DA * h + 1:DA * h + DA],
                                                 v_ps[:, D * h:D * h + D],
                                                 bv_bc[:, D * h:D * h + D])
                        # ones columns at local col 0 of each head's group, so
                        # the denominator row of y_aug^T lands at partition 0
                        # (matmul moving operands must start at partition
                        # 0/32/64, and the DVE divide needs aligned bases)
                        nc.vector.tensor_copy(vA[tb][:, 0::DA], ones8_f)

                # W_proj loads into the space freed by the wv/xl pools
                wp_p = tc.alloc_tile_pool(name="wp", bufs=1)
                bp_bc = wp_p.tile([P, C], F32, name="bp_bc", tag="bpbc", bufs=1)
                nc.sync.dma_start(
                    bp_bc[:], bp_d.unsqueeze(0).partition_broadcast(P).squeeze(1))
                wp = []
                for cb in range(NCB):
                    w = wp_p.tile([P, C], BF16, name=f"wp{cb}")
                    nc.sync.dma_start(w[:], wp_d[cb * P:(cb + 1) * P, :])
                    wp.append(w)

                # ---- per-head pipelined attention ----
                qk_sb = {}

                def emit_qk_part(h, part):
                    # part 0: q projection (both 512-halves), parts 1/2: k
                    # halves. Emitted interleaved between head h-1's S blocks.
                    if part == 0:
                        qk_sb[h] = (qk_p.tile([D, T], BF16, name="qT", tag="qkt"),
                                    qk_p.tile([D, T], BF16, name="kT", tag="qkt"))
                    qT, kT = qk_sb[h]
                    jts = (0, 1) if part == 0 else (part - 1,)
                    dst = qT if part == 0 else kT
                    off = D * h if part == 0 else C + D * h
                    bcol = b_qk[:, h:h + 1] if part == 0 else b_qk[:, 8 + h:9 + h]
                    for jt in jts:
                        sl = slice(jt * 512, (jt + 1) * 512)
                        qk_ps = ps.tile([D, 512], F32, name="qk_ps", tag="big", bufs=3)
                        for cb in range(NCB):
                            nc.tensor.matmul(qk_ps[:], wqk[cb][:, off:off + D],
                                             xT[cb][:, sl],
                                             start=(cb == 0), stop=(cb == NCB - 1))
                        nc.vector.tensor_scalar_add(dst[:, sl], qk_ps[:], bcol)

                def emit_S(h, ib):
                    q0 = P * ib
                    qT, kT = qk_sb[h]
                    s_ps = ps.tile([P, T], F32, name="s_ps", tag="big", bufs=3)
                    kblk = kT[:, ib * P:(ib + 1) * P]
                    if q0 < 512:
                        nc.tensor.matmul(s_ps[:, q0:512], kblk,
                                         qT[:, q0:512], start=True, stop=True)
                    r0 = max(q0, 512)
                    nc.tensor.matmul(s_ps[:, r0:T], kblk,
                                     qT[:, r0:T], start=True, stop=True)
                    p_t = p_p.tile([P, T], BF16, name="p_t")
                    nc.scalar.activation(p_t[:, q0:T], s_ps[:, q0:T],
                                         Exp, scale=SCALE)
                    # zero the upper triangle of the diagonal 128-col block
                    nc.vector.tensor_mul(p_t[:, q0:q0 + P],
                                         p_t[:, q0:q0 + P], tri)
                    return p_t

                def emit_tail(h, y_sbs):
                    # y^T = y_aug^T[0:96] / broadcast(denominator row).
                    # Heads 6/7 are on the critical path into the projection:
                    # broadcast via a ones-column PE matmul. Earlier heads go
                    # through DRAM (hidden under compute).
                    pe_bc = h >= 6
                    for half, y_sb in ((0, y_sbs[0]), (1, y_sbs[1])):
                        q_sl = slice(half * 512, (half + 1) * 512)
                        # y_sb row 0 = denominator, rows 1:97 = y^T. The DVE
                        # has no divide (invalid ISA), so reciprocal via the
                        # shared Ln/Exp activation table, then multiply.
                        ln_s = sm_p.tile([1, 512], F32, name="ln_s",
                                         tag="lns", bufs=2)
                        rc_b = sm_p.tile([1, 512], BF16, name="rc_b",
                                         tag="rcb", bufs=2)
                        nc.scalar.activation(ln_s[:], y_sb[0:1, :], Ln)
                        nc.scalar.activation(rc_b[:], ln_s[:], Exp, scale=-1.0)
                        y_n = sm_p.tile([DA, 512], BF16, name="y_n", tag="yn", bufs=2)
                        if pe_bc:
                            bc_ps = ps.tile([DA, 512], F32, name="bc_ps",
                                            tag="big", bufs=3)
                            nc.tensor.matmul(bc_ps[:], ones97, rc_b[:],
                                             start=True, stop=True)
                            nc.vector.tensor_mul(y_n[:], y_sb[:], bc_ps[:])
                        else:
                            row = den_dram[2 * h + half:2 * h + half + 1, :]
                            nc.sync.dma_start(row, rc_b[:])
                            den_bc = sm_p.tile([DA, 512], BF16, name="den_bc",
                                               tag="bcsb", bufs=2)
                            nc.sync.dma_start(den_bc[:],
                                              row.partition_broadcast(DA).squeeze(1))
                            nc.vector.tensor_mul(y_n[:], y_sb[:], den_bc[:])
                        # scatter head rows into the feature-packed yT tiles
                        # (partition shift -> must go through DMA)
                        f0 = D * h
                        while f0 < D * (h + 1):
                            cb2, b0 = f0 // P, f0 % P
                            seg = min(P - b0, D * (h + 1) - f0)
                            nc.sync.dma_start(
                                yTp[cb2][b0:b0 + seg, q_sl],
                                y_n[1 + f0 - D * h:1 + f0 - D * h + seg, :])
                            f0 += seg

                # split projection: token blocks 0-3 accumulate cb0-4 inside
                # head 7's cycle (into 512/256 PSUM halves), bias-added into
                # SBUF partials; the cb5 contribution is added in the epilogue
                o_part = []

                def emit_proj_partial(tb):
                    o_lo = ps.tile([P, 512], F32, name="o_lo", tag="yps", bufs=2)
                    o_hi = ps.tile([P, 256], F32, name="o_hi", tag="yps", bufs=2)
                    for cb in range(NCB - 1):
                        nc.tensor.matmul(o_lo[:], yTp[cb][:, tb * P:(tb + 1) * P],
                                         wp[cb][:, 0:512],
                                         start=(cb == 0), stop=(cb == NCB - 2))
                    for cb in range(NCB - 1):
                        nc.tensor.matmul(o_hi[:], yTp[cb][:, tb * P:(tb + 1) * P],
                                         wp[cb][:, 512:C],
                                         start=(cb == 0), stop=(cb == NCB - 2))
                    o_sb = o_p.tile([P, C], F32, name=f"o_part{tb}", tag="opart",
                                    bufs=4)
                    nc.vector.tensor_add(o_sb[:, 0:512], o_lo[:], bp_bc[:, 0:512])
                    nc.vector.tensor_add(o_sb[:, 512:C], o_hi[:], bp_bc[:, 512:C])
                    o_part.append(o_sb)

                ysb_hist = {}
                for part in range(3):
                    emit_qk_part(0, part)
                for h in range(H):
                    if h == 7:
                        emit_tail(6, ysb_hist.pop(6))
                    ptiles = []
                    fill = 0
                    for ib in range(NT):
                        ptiles.append(emit_S(h, ib))
                        if ib % 2 == 1 and fill < (4 if h == 7 else 3):
                            if h < 7:
                                emit_qk_part(h + 1, fill)
                            else:
                                emit_proj_partial(fill)
                            fill += 1
                    y_l = ps.tile([DA, 512], F32, name="y_l", tag="yps", bufs=2)
                    y_r = ps.tile([DA, 512], F32, name="y_r", tag="yps", bufs=2)
                    for ib in range(NT):
                        q0 = P * ib
                        va = vA[ib][:, DA * h:DA * h + DA]
                        if q0 < 512:
                            nc.tensor.matmul(y_l[:, q0:512], va,
                                             ptiles[ib][:, q0:512],
                                             start=(ib == 0), stop=(ib == 3))
                            nc.tensor.matmul(y_r[:], va, ptiles[ib][:, 512:T],
                                             start=(ib == 0), stop=False)
                        else:
                            nc.tensor.matmul(y_r[:, q0 - 512:512], va,
                                             ptiles[ib][:, q0:T],
                                             start=False, stop=(ib == NT - 1))
                    # tails for heads 0-5 emitted one head late, after this
                    # head's PV matmuls (their DMA round trip must not block
                    # the DVE queue ahead of the q/k copybacks)
                    if 1 <= h <= 6:
                        emit_tail(h - 1, ysb_hist.pop(h - 1))
                    y_sbs = []
                    for y_ps in (y_l, y_r):
                        y_sb = sm_p.tile([DA, 512], BF16, name="y_sb", tag="ysb", bufs=3)
                        nc.vector.tensor_copy(y_sb[:], y_ps[:])
                        y_sbs.append(y_sb)
                    ysb_hist[h] = y_sbs
                emit_tail(7, ysb_hist.pop(7))

                # ---------------- projection epilogue ----------------
                # cover head 7's tail with tb4's cb0-4 matmuls, then finish
                # the cb5 contributions for tb0-3, then tb5-7 full passes
                def emit_proj_full(tb, skip_last=False):
                    o_ps = ps.tile([P, C], F32, name="o_ps", tag="big", bufs=3)
                    last = NCB - 1 if not skip_last else NCB - 2
                    for cb in range(NCB):
                        if skip_last and cb == NCB - 1:
                            continue
                        nc.tensor.matmul(o_ps[:, 0:512],
                                         yTp[cb][:, tb * P:(tb + 1) * P],
                                         wp[cb][:, 0:512],
                                         start=(cb == 0), stop=(cb == last))
                    for cb in range(NCB):
                        if skip_last and cb == NCB - 1:
                            continue
                        nc.tensor.matmul(o_ps[:, 512:C],
                                         yTp[cb][:, tb * P:(tb + 1) * P],
                                         wp[cb][:, 512:C],
                                         start=(cb == 0), stop=(cb == last))
                    return o_ps

                def emit_cb5(tb):
                    cb = NCB - 1
                    o_lo = ps.tile([P, 512], F32, name="c5_lo", tag="yps", bufs=2)
                    o_hi = ps.tile([P, 256], F32, name="c5_hi", tag="yps", bufs=2)
                    nc.tensor.matmul(o_lo[:], yTp[cb][:, tb * P:(tb + 1) * P],
                                     wp[cb][:, 0:512], start=True, stop=True)
                    nc.tensor.matmul(o_hi[:], yTp[cb][:, tb * P:(tb + 1) * P],
                                     wp[cb][:, 512:C], start=True, stop=True)
                    o_sb = o_part[tb]
                    nc.vector.tensor_add(o_sb[:, 0:512], o_sb[:, 0:512], o_lo[:])
                    nc.vector.tensor_add(o_sb[:, 512:C], o_sb[:, 512:C], o_hi[:])
                    nc.sync.dma_start(out_d[tb * P:(tb + 1) * P, :], o_sb[:])

                held = emit_proj_full(4, skip_last=True)
                for tb in range(4):
                    emit_cb5(tb)
                # finish tb4: cb5 into the held accumulation
                cb = NCB - 1
                nc.tensor.matmul(held[:, 0:512], yTp[cb][:, 4 * P:5 * P],
                                 wp[cb][:, 0:512], start=False, stop=True)
                nc.tensor.matmul(held[:, 512:C], yTp[cb][:, 4 * P:5 * P],
                                 wp[cb][:, 512:C], start=False, stop=True)
                o_sb = o_p.tile([P, C], F32, name="o_sb")
                nc.vector.tensor_add(o_sb[:, 0:512], held[:, 0:512], bp_bc[:, 0:512])
                nc.sync.dma_start(out_d[4 * P:5 * P, 0:512], o_sb[:, 0:512])
                nc.vector.tensor_add(o_sb[:, 512:C], held[:, 512:C], bp_bc[:, 512:C])
                nc.sync.dma_start(out_d[4 * P:5 * P, 512:C], o_sb[:, 512:C])
                for tb in range(5, NT):
                    o_ps = emit_proj_full(tb)
                    o_sb = o_p.tile([P, C], F32, name="o_sb")
                    nc.vector.tensor_add(o_sb[:, 0:512], o_ps[:, 0:512],
                                         bp_bc[:, 0:512])
                    nc.sync.dma_start(out_d[tb * P:(tb + 1) * P, 0:512],
                                      o_sb[:, 0:512])
                    nc.vector.tensor_add(o_sb[:, 512:C], o_ps[:, 512:C],
                                         bp_bc[:, 512:C])
                    nc.sync.dma_start(out_d[tb * P:(tb + 1) * P, 512:C],
                                      o_sb[:, 512:C])
                wp_p.release()

    nc.compile()
    return nc


def run(inputs, trace=False):
    import ml_dtypes
    import concourse.bass_utils as bass_utils

    nc = _CACHE.get("nc")
    if nc is None:
        nc = _CACHE["nc"] = _build()

    bf16 = ml_dtypes.bfloat16
    x = np.ascontiguousarray(inputs["x"]).astype(bf16)
    wa = np.ascontiguousarray(inputs["W_attn"]).astype(bf16)
    ba = np.ascontiguousarray(inputs["b_attn"], dtype=np.float32)
    wp = np.ascontiguousarray(inputs["W_proj"]).astype(bf16)
    bp = np.ascontiguousarray(inputs["b_proj"], dtype=np.float32)
    aux_c = np.zeros((P, 3 * P), dtype=bf16)
    aux_c[:, 0:P] = np.eye(P, dtype=np.float32)
    aux_c[:, P:2 * P] = np.triu(np.ones((P, P), dtype=np.float32))
    aux_c[:, 2 * P:3 * P] = 1.0
    aux_b = np.ascontiguousarray(ba[:16 * D].reshape(16, D).T)
    B = x.shape[0]
    in_maps = [
        {"x": np.ascontiguousarray(x[b]), "W_attn": wa, "b_attn": ba,
         "W_proj": wp, "b_proj": bp, "aux_c": aux_c, "aux_b": aux_b}
        for b in range(B)
    ]
    res = bass_utils.run_bass_kernel_spmd(
        nc, in_maps, core_ids=list(range(B)), trace=trace)
    out = np.stack([r["out"] for r in res.results], axis=0)
    return out, res


def kernel(**inputs):
    out, _ = run(inputs, trace=False)
    return out


# revision 15
# speedup vs baseline: 1.0597x; 1.0597x over previous
"""Causal self-attention (B=8, T=1024, C=768, H=8 heads) for 8 TRN2 NeuronCores.

Strategy: pure data parallelism — one batch element per core. Each core runs an
identical Bass/Tile program computing the full attention block for its batch
element; weights are replicated. No collectives.

v3 (from trace analysis of v2, 164.0us):
  - Host-provided constants (identity / triangle mask / ones, and the q,k bias
    columns pre-rearranged): removes the gpsimd setup chain that delayed the
    first tensor op to 7.8us, and the 1536-descriptor bias scatter DMA.
  - PE warm-up matmuls start right after the constant DMA (~0.5us in), so the
    HAM clock gate (K=4/8 -> 1.2 GHz cold) opens before the real work.
  - Per-head software pipeline: the 8 S^T blocks of head h are interleaved
    with the q/k projection matmuls of head h+1, so the serial exp chain on
    the ACT engine (~6us/head) hides under PE work instead of pacing it
    (v2 ran a 12.7us/head cadence for 9.4us of PE work).
  - Softmax tails: reciprocal via Ln/Exp on the shared activation table
    (DVE divide is invalid ISA), multiplied on the DVE.
  - Heads 6/7 broadcast the denominator across partitions with a ones-column
    PE matmul instead of the DRAM round trip; head 6's tail gates the
    early output-projection blocks emitted inside head 7's cycle.
  - Output projection split: contraction blocks cb0-4 of token blocks 0-3 run
    inside head 7's cycle (covering its exp chain); the cb5 contribution
    (which needs head 7's result) lands in a second accumulation pass.

Per-core pipeline:
  1. x [T,C] -> x^T [C,T] via PE transposes, bf16.
  2. v = x @ W_v + b_v in [token, feat] layout as v_aug [128, 8*97] with a
     ones column per head (the P@V matmul then also produces the softmax
     denominator).
  3. Per head: q^T, k^T = (x @ W_{q,k} + b)^T [96, 1024]; S^T[tk,q] =
     k^T.T @ q^T per 128-key block over the causally-valid query range;
     P = exp(S*scale) masked by a DVE triangle multiply; y_aug^T = V_aug^T P.
  4. y^T = y_aug^T[0:96] / broadcast(denominator); out = y @ W_proj + b_proj.
"""
import sys

sys.path.insert(0, "/opt/trn_rl_repo")

import numpy as np

T, C, H, D = 1024, 768, 8, 96
C3 = 3 * C
P = 128
NT = T // P   # 8 token blocks
NCB = C // P  # 6 feature blocks
DA = D + 1    # 97: head dim + denominator column

_CACHE = {}


def _build():
    import concourse.bacc as bacc
    import concourse.mybir as mybir
    import concourse.tile as tile

    F32 = mybir.dt.float32
    BF16 = mybir.dt.bfloat16
    Exp = mybir.ActivationFunctionType.Exp
    Ln = mybir.ActivationFunctionType.Ln
    SCALE = 1.0 / float(np.sqrt(D))

    nc = bacc.Bacc("TRN2", target_bir_lowering=False, debug=False, num_devices=8)
    x_d = nc.dram_tensor("x", [T, C], BF16, kind="ExternalInput").ap()
    wa_d = nc.dram_tensor("W_attn", [C, C3], BF16, kind="ExternalInput").ap()
    ba_d = nc.dram_tensor("b_attn", [C3], F32, kind="ExternalInput").ap()
    wp_d = nc.dram_tensor("W_proj", [C, C], BF16, kind="ExternalInput").ap()
    bp_d = nc.dram_tensor("b_proj", [C], F32, kind="ExternalInput").ap()
    # host-prepared constants: [identity | triangle mask | ones]
    auxc_d = nc.dram_tensor("aux_c", [P, 3 * P], BF16, kind="ExternalInput").ap()
    auxb_d = nc.dram_tensor("aux_b", [D, 16], F32, kind="ExternalInput").ap()
    out_d = nc.dram_tensor("out", [T, C], F32, kind="ExternalOutput").ap()

    with tile.TileContext(nc) as tc:
        with tc.tile_pool(name="const", bufs=1) as const_p, \
             tc.tile_pool(name="vp", bufs=1) as v_p, \
             tc.tile_pool(name="qkt", bufs=5) as qk_p, \
             tc.tile_pool(name="yt", bufs=1) as yT_p, \
             tc.tile_pool(name="sm", bufs=2) as sm_p, \
             tc.tile_pool(name="ob", bufs=2) as o_p, \
             tc.tile_pool(name="pp", bufs=8) as p_p, \
             tc.tile_pool(name="ps", bufs=1, space="PSUM") as ps:
            constF = const_p.tile([P, 3 * P], BF16, name="constF")
            ident = constF[:, 0:P]
            tri = constF[:, P:2 * P]
            ones8_f = constF[:, 2 * P:2 * P + H]
            ones97 = constF[0:1, 2 * P + H:2 * P + H + DA]
            b_qk = const_p.tile([D, 16], F32, name="b_qk")
            nc.sync.dma_start(constF[:], auxc_d)
            nc.sync.dma_start(b_qk[:], auxb_d)
            bv_bc = const_p.tile([P, C], F32, name="bv_bc")
            vA = [v_p.tile([P, DA * H], BF16, name=f"vA{t}") for t in range(NT)]
            yTp = [yT_p.tile([P, T], BF16, name=f"yTp{cb}") for cb in range(NCB)]
            # DRAM staging for the denominator rows (SBUF->SBUF DMA cannot do
            # 0-stride broadcast, DRAM->SBUF can); heads 0-5 only
            den_dram = nc.dram_tensor("den_stage", [12, 512], BF16,
                                      kind="Internal").ap()

            with tc.tile_pool(name="xT", bufs=1) as xT_p, \
                 tc.tile_pool(name="wqk", bufs=1) as wqk_p:
                xT = [xT_p.tile([P, T], BF16, name=f"xT{cb}") for cb in range(NCB)]

                # ---- x^T transposes + v projection (scoped W_v / x loads) ----
                wqk = []
                with tc.tile_pool(name="xl", bufs=8) as x_p, \
                     tc.tile_pool(name="wv", bufs=1) as wv_p:
                    # issue every load up front; they stream on parallel DMA
                    # queues while the PE warm-up runs
                    x_ts = []
                    for tb in range(NT):
                        x_t = x_p.tile([P, C], BF16, name="x_t")
                        nc.sync.dma_start(x_t[:], x_d[tb * P:(tb + 1) * P, :])
                        x_ts.append(x_t)
                    wv = []
                    for cb in range(NCB):
                        w = wv_p.tile([P, C], BF16, name=f"wv{cb}")
                        nc.sync.dma_start(w[:], wa_d[cb * P:(cb + 1) * P,
                                                     2 * C:3 * C])
                        wv.append(w)
                    for cb in range(NCB):
                        w = wqk_p.tile([P, 2 * C], BF16, name=f"wqk{cb}")
                        nc.sync.dma_start(w[:], wa_d[cb * P:(cb + 1) * P, 0:2 * C])
                        wqk.append(w)

                    # PE warm-up: the HAM clock gate defaults to K=4/8
                    # (1.2 GHz) and needs ~3.4us of sustained matmul activity
                    # to open. Burn the x-DMA wait on dummy matmuls so the
                    # real transposes start at full clock.
                    warm_ps = ps.tile([P, 2 * P], F32, name="warm_ps",
                                      tag="big", bufs=3)
                    for _ in range(10):
                        nc.tensor.matmul(warm_ps[:], ident, constF[:, 0:2 * P],
                                         start=True, stop=True)

                    for jt in range(2):
                        for cb in range(NCB):
                            tr_ps = ps.tile([P, 512], BF16, name="tr_ps", tag="big", bufs=3)
                            for k in range(4):
                                nc.tensor.transpose(tr_ps[:, k * P:(k + 1) * P],
                                                    x_ts[4 * jt + k][:, cb * P:(cb + 1) * P],
                                                    ident)
                            nc.vector.tensor_copy(xT[cb][:, jt * 512:(jt + 1) * 512],
                                                  tr_ps[:])

                    nc.sync.dma_start(
                        bv_bc[:],
                        ba_d.unsqueeze(0)[:, 2 * C:3 * C].partition_broadcast(P).squeeze(1))
                    for tb in range(NT):
                        v_ps = ps.tile([P, C], F32, name="v_ps", tag="big", bufs=3)
                        for cb in range(NCB):
                            lhsT = xT[cb][:, tb * P:(tb + 1) * P]
                            nc.tensor.matmul(v_ps[:, 0:512], lhsT, wv[cb][:, 0:512],
                                             start=(cb == 0), stop=(cb == NCB - 1))
                            nc.tensor.matmul(v_ps[:, 512:C], lhsT, wv[cb][:, 512:C],
                                             start=(cb == 0), stop=(cb == NCB - 1))
                        for h in range(H):
                            nc.vector.tensor_add(vA[tb][:, DA * h + 1:DA * h + DA],
                                                 v_ps[:, D * h:D * h + D],
                                                 bv_bc[:, D * h:D * h + D])
                        # ones columns at local col 0 of each head's group, so
                        # the denominator row of y_aug^T lands at partition 0
                        # (matmul moving operands must start at partition
                        # 0/32/64, and the DVE divide needs aligned bases)
                        nc.vector.tensor_copy(vA[tb][:, 0::DA], ones8_f)

                # W_proj loads into the space freed by the wv/xl pools
                wp_p = tc.alloc_tile_pool(name="wp", bufs=1)
                bp_bc = wp_p.tile([P, C], F32, name="bp_bc", tag="bpbc", bufs=1)
                nc.sync.dma_start(
                    bp_bc[:], bp_d.unsqueeze(0).partition_broadcast(P).squeeze(1))
                wp = []
                for cb in range(NCB):
                    w = wp_p.tile([P, C], BF16, name=f"wp{cb}")
                    nc.sync.dma_start(w[:], wp_d[cb * P:(cb + 1) * P, :])
                    wp.append(w)

                # ---- per-head pipelined attention ----
                qk_sb = {}

                def emit_qk_part(h, part):
                    # part 0: q projection (both 512-halves), parts 1/2: k
                    # halves. Emitted interleaved between head h-1's S blocks.
                    if part == 0:
                        qk_sb[h] = (qk_p.tile([D, T], BF16, name="qT", tag="qkt"),
                                    qk_p.tile([D, T], BF16, name="kT", tag="qkt"))
                    qT, kT = qk_sb[h]
                    jts = (0, 1) if part == 0 else (part - 1,)
                    dst = qT if part == 0 else kT
                    off = D * h if part == 0 else C + D * h
                    bcol = b_qk[:, h:h + 1] if part == 0 else b_qk[:, 8 + h:9 + h]
                    for jt in jts:
                        sl = slice(jt * 512, (jt + 1) * 512)
                        qk_ps = ps.tile([D, 512], F32, name="qk_ps", tag="big", bufs=3)
                        for cb in range(NCB):
                            nc.tensor.matmul(qk_ps[:], wqk[cb][:, off:off + D],
                                             xT[cb][:, sl],
                                             start=(cb == 0), stop=(cb == NCB - 1))
                        nc.vector.tensor_scalar_add(dst[:, sl], qk_ps[:], bcol)

                def emit_S(h, ib):
                    q0 = P * ib
                    qT, kT = qk_sb[h]
                    s_ps = ps.tile([P, T], F32, name="s_ps", tag="big", bufs=3)
                    kblk = kT[:, ib * P:(ib + 1) * P]
                    if q0 < 512:
                        nc.tensor.matmul(s_ps[:, q0:512], kblk,
                                         qT[:, q0:512], start=True, stop=True)
                    r0 = max(q0, 512)
                    nc.tensor.matmul(s_ps[:, r0:T], kblk,
                                     qT[:, r0:T], start=True, stop=True)
                    p_t = p_p.tile([P, T], BF16, name="p_t")
                    nc.scalar.activation(p_t[:, q0:T], s_ps[:, q0:T],
                                         Exp, scale=SCALE)
                    # zero the upper triangle of the diagonal 128-col block
                    nc.vector.tensor_mul(p_t[:, q0:q0 + P],
                                         p_t[:, q0:q0 + P], tri)
                    return p_t

                def emit_tail(h, y_sbs):
                    # y^T = y_aug^T[0:96] / broadcast(denominator row).
                    # Heads 6/7 are on the critical path into the projection:
                    # broadcast via a ones-column PE matmul. Earlier heads go
                    # through DRAM (hidden under compute).
                    pe_bc = h >= 6
                    for half, y_sb in ((0, y_sbs[0]), (1, y_sbs[1])):
                        q_sl = slice(half * 512, (half + 1) * 512)
                        # y_sb row 0 = denominator, rows 1:97 = y^T. The DVE
                        # has no divide (invalid ISA), so reciprocal via the
                        # shared Ln/Exp activation table, then multiply.
                        ln_s = sm_p.tile([1, 512], F32, name="ln_s",
                                         tag="lns", bufs=2)
                        rc_b = sm_p.tile([1, 512], BF16, name="rc_b",
                                         tag="rcb", bufs=2)
                        nc.scalar.activation(ln_s[:], y_sb[0:1, :], Ln)
                        nc.scalar.activation(rc_b[:], ln_s[:], Exp, scale=-1.0)
                        y_n = sm_p.tile([DA, 512], BF16, name="y_n", tag="yn", bufs=2)
                        if pe_bc:
                            bc_ps = ps.tile([DA, 512], F32, name="bc_ps",
                                            tag="big", bufs=3)
                            nc.tensor.matmul(bc_ps[:], ones97, rc_b[:],
                                             start=True, stop=True)
                            nc.vector.tensor_mul(y_n[:], y_sb[:], bc_ps[:])
                        else:
                            row = den_dram[2 * h + half:2 * h + half + 1, :]
                            nc.sync.dma_start(row, rc_b[:])
                            den_bc = sm_p.tile([DA, 512], BF16, name="den_bc",
                                               tag="bcsb", bufs=2)
                            nc.sync.dma_start(den_bc[:],
                                              row.partition_broadcast(DA).squeeze(1))
                            nc.vector.tensor_mul(y_n[:], y_sb[:], den_bc[:])
                        # scatter head rows into the feature-packed yT tiles
                        # (partition shift -> must go through DMA)
                        f0 = D * h
                        while f0 < D * (h + 1):
                            cb2, b0 = f0 // P, f0 % P
                            seg = min(P - b0, D * (h + 1) - f0)
                            nc.sync.dma_start(
                                yTp[cb2][b0:b0 + seg, q_sl],
                                y_n[1 + f0 - D * h:1 + f0 - D * h + seg, :])
                            f0 += seg

                # split projection: token blocks 0-3 accumulate cb0-4 inside
                # head 7's cycle (into 512/256 PSUM halves), bias-added into
                # SBUF partials; the cb5 contribution is added in the epilogue
                o_part = []

                def emit_proj_partial(tb):
                    o_lo = ps.tile([P, 512], F32, name="o_lo", tag="yps", bufs=2)
                    o_hi = ps.tile([P, 256], F32, name="o_hi", tag="yps", bufs=2)
                    for cb in range(NCB - 1):
                        nc.tensor.matmul(o_lo[:], yTp[cb][:, tb * P:(tb + 1) * P],
                                         wp[cb][:, 0:512],
                                         start=(cb == 0), stop=(cb == NCB - 2))
                    for cb in range(NCB - 1):
                        nc.tensor.matmul(o_hi[:], yTp[cb][:, tb * P:(tb + 1) * P],
                                         wp[cb][:, 512:C],
                                         start=(cb == 0), stop=(cb == NCB - 2))
                    o_sb = o_p.tile([P, C], F32, name=f"o_part{tb}", tag="opart",
                                    bufs=4)
                    nc.vector.tensor_add(o_sb[:, 0:512], o_lo[:], bp_bc[:, 0:512])
                    nc.vector.tensor_add(o_sb[:, 512:C], o_hi[:], bp_bc[:, 512:C])
                    o_part.append(o_sb)

                ysb_hist = {}
                for part in range(3):
                    emit_qk_part(0, part)
                for h in range(H):
                    if h == 7:
                        emit_tail(6, ysb_hist.pop(6))
                    ptiles = []
                    fill = 0
                    for ib in range(NT):
                        ptiles.append(emit_S(h, ib))
                        if ib % 2 == 1 and fill < (4 if h == 7 else 3):
                            if h < 7:
                                emit_qk_part(h + 1, fill)
                            else:
                                emit_proj_partial(fill)
                            fill += 1
                    y_l = ps.tile([DA, 512], F32, name="y_l", tag="yps", bufs=2)
                    y_r = ps.tile([DA, 512], F32, name="y_r", tag="yps", bufs=2)
                    for ib in range(NT):
                        q0 = P * ib
                        va = vA[ib][:, DA * h:DA * h + DA]
                        if q0 < 512:
                            nc.tensor.matmul(y_l[:, q0:512], va,
                                             ptiles[ib][:, q0:512],
                                             start=(ib == 0), stop=(ib == 3))
                            nc.tensor.matmul(y_r[:], va, ptiles[ib][:, 512:T],
                                             start=(ib == 0), stop=False)
                        else:
                            nc.tensor.matmul(y_r[:, q0 - 512:512], va,
                                             ptiles[ib][:, q0:T],
                                             start=False, stop=(ib == NT - 1))
                    # tails for heads 0-5 emitted one head late, after this
                    # head's PV matmuls (their DMA round trip must not block
                    # the DVE queue ahead of the q/k copybacks)
                    if 1 <= h <= 6:
                        emit_tail(h - 1, ysb_hist.pop(h - 1))
                    y_sbs = []
                    for y_ps in (y_l, y_r):
                        y_sb = sm_p.tile([DA, 512], BF16, name="y_sb", tag="ysb", bufs=3)
                        nc.vector.tensor_copy(y_sb[:], y_ps[:])
                        y_sbs.append(y_sb)
                    ysb_hist[h] = y_sbs
                emit_tail(7, ysb_hist.pop(7))

                # ---------------- projection epilogue ----------------
                # cover head 7's tail with tb4's cb0-4 matmuls, then finish
                # the cb5 contributions for tb0-3, then tb5-7 full passes
                def emit_proj_full(tb, skip_last=False):
                    o_ps = ps.tile([P, C], F32, name="o_ps", tag="big", bufs=3)
                    last = NCB - 1 if not skip_last else NCB - 2
                    for cb in range(NCB):
                        if skip_last and cb == NCB - 1:
                            continue
                        nc.tensor.matmul(o_ps[:, 0:512],
                                         yTp[cb][:, tb * P:(tb + 1) * P],
                                         wp[cb][:, 0:512],
                                         start=(cb == 0), stop=(cb == last))
                    for cb in range(NCB):
                        if skip_last and cb == NCB - 1:
                            continue
                        nc.tensor.matmul(o_ps[:, 512:C],
                                         yTp[cb][:, tb * P:(tb + 1) * P],
                                         wp[cb][:, 512:C],
                                         start=(cb == 0), stop=(cb == last))
                    return o_ps

                def emit_cb5(tb):
                    cb = NCB - 1
                    o_lo = ps.tile([P, 512], F32, name="c5_lo", tag="yps", bufs=2)
                    o_hi = ps.tile([P, 256], F32, name="c5_hi", tag="yps", bufs=2)
                    nc.tensor.matmul(o_lo[:], yTp[cb][:, tb * P:(tb + 1) * P],
                                     wp[cb][:, 0:512], start=True, stop=True)
                    nc.tensor.matmul(o_hi[:], yTp[cb][:, tb * P:(tb + 1) * P],
                                     wp[cb][:, 512:C], start=True, stop=True)
                    o_sb = o_part[tb]
                    nc.vector.tensor_add(o_sb[:, 0:512], o_sb[:, 0:512], o_lo[:])
                    nc.vector.tensor_add(o_sb[:, 512:C], o_sb[:, 512:C], o_hi[:])
                    nc.sync.dma_start(out_d[tb * P:(tb + 1) * P, :], o_sb[:])

                held = emit_proj_full(4, skip_last=True)
                for tb in range(4):
                    emit_cb5(tb)
                # finish tb4: cb5 into the held accumulation
                cb = NCB - 1
                nc.tensor.matmul(held[:, 0:512], yTp[cb][:, 4 * P:5 * P],
                                 wp[cb][:, 0:512], start=False, stop=True)
                nc.tensor.matmul(held[:, 512:C], yTp[cb][:, 4 * P:5 * P],
                                 wp[cb][:, 512:C], start=False, stop=True)
                o_sb = o_p.tile([P, C], F32, name="o_sb")
                nc.vector.tensor_add(o_sb[:, 0:512], held[:, 0:512], bp_bc[:, 0:512])
                nc.sync.dma_start(out_d[4 * P:5 * P, 0:512], o_sb[:, 0:512])
                nc.vector.tensor_add(o_sb[:, 512:C], held[:, 512:C], bp_bc[:, 512:C])
                nc.sync.dma_start(out_d[4 * P:5 * P, 512:C], o_sb[:, 512:C])
                for tb in range(5, NT):
                    o_ps = emit_proj_full(tb)
                    o_sb = o_p.tile([P, C], F32, name="o_sb")
                    nc.vector.tensor_add(o_sb[:, 0:512], o_ps[:, 0:512],
                                         bp_bc[:, 0:512])
                    nc.sync.dma_start(out_d[tb * P:(tb + 1) * P, 0:512],
                                      o_sb[:, 0:512])
                    nc.vector.tensor_add(o_sb[:, 512:C], o_ps[:, 512:C],
                                         bp_bc[:, 512:C])
                    nc.sync.dma_start(out_d[tb * P:(tb + 1) * P, 512:C],
                                      o_sb[:, 512:C])
                wp_p.release()

    # The act-table-load pass assigns each activation the first table set
    # containing its function, which puts Exp in exp_and_others and Ln in
    # natural_log — a 1.3us table reload on every switch. All our functions
    # (Exp, Ln, Identity) live together in natural_log_exp_and_others, so
    # hide the other sets (keeping dict order — act_func_set_id is positional)
    # during this build.
    import concourse.hw_specs as hw_specs
    orig_tables = hw_specs.get_activation_tables

    def _tables(arch, *a, **kw):
        tabs = orig_tables(arch, *a, **kw)
        pref = "natural_log_exp_and_others"
        if pref not in tabs:
            return tabs
        return {k: (v if k == pref else type(v)()) for k, v in tabs.items()}

    import concourse.bacc as bacc_mod
    hw_specs.get_activation_tables = _tables
    bacc_orig = getattr(bacc_mod, "get_activation_tables", None)
    try:
        if bacc_orig is not None:
            bacc_mod.get_activation_tables = _tables
        nc.compile()
    finally:
        hw_specs.get_activation_tables = orig_tables
        if bacc_orig is not None:
            bacc_mod.get_activation_tables = bacc_orig
    return nc


def run(inputs, trace=False):
    import ml_dtypes
    import concourse.bass_utils as bass_utils

    nc = _CACHE.get("nc")
    if nc is None:
        nc = _CACHE["nc"] = _build()

    bf16 = ml_dtypes.bfloat16
    x = np.ascontiguousarray(inputs["x"]).astype(bf16)
    wa = np.ascontiguousarray(inputs["W_attn"]).astype(bf16)
    ba = np.ascontiguousarray(inputs["b_attn"], dtype=np.float32)
    wp = np.ascontiguousarray(inputs["W_proj"]).astype(bf16)
    bp = np.ascontiguousarray(inputs["b_proj"], dtype=np.float32)
    aux_c = np.zeros((P, 3 * P), dtype=bf16)
    aux_c[:, 0:P] = np.eye(P, dtype=np.float32)
    aux_c[:, P:2 * P] = np.triu(np.ones((P, P), dtype=np.float32))
    aux_c[:, 2 * P:3 * P] = 1.0
    aux_b = np.ascontiguousarray(ba[:16 * D].reshape(16, D).T)
    B = x.shape[0]
    in_maps = [
        {"x": np.ascontiguousarray(x[b]), "W_attn": wa, "b_attn": ba,
         "W_proj": wp, "b_proj": bp, "aux_c": aux_c, "aux_b": aux_b}
        for b in range(B)
    ]
    res = bass_utils.run_bass_kernel_spmd(
        nc, in_maps, core_ids=list(range(B)), trace=trace)
    out = np.stack([r["out"] for r in res.results], axis=0)
    return out, res


def kernel(**inputs):
    out, _ = run(inputs, trace=False)
    return out
